# revision 16
# baseline (speedup 1.0000x reference)
"""BankedLinear (MoE-style banked linear) Trainium2 kernel.

Reference computation (per token t, with k=2 selected banks):
    out[t] = sum_k prob[t,k] * (x[t] @ W[sel[t,k]] + bias[sel[t,k]])

Strategy (expert-parallel over 8 NeuronCores):
  - Core c owns banks [8c, 8c+8).  Its weight slab is the dominant,
    unavoidable HBM traffic; each bank is read exactly once system-wide.
  - Host routes token-bank pairs to cores by selected bank, pre-scales each
    gathered token row by its probability, transposes to [in_feature, slot],
    and pads to CAP=32 slots per bank.
  - Precision: harness tolerance is rel_err < 2e-2; a single bf16 matmul
    term (x_bf16 @ W_bf16 accumulated in fp32 PSUM) gives ~3e-3, so weights
    and activations ship as plain bf16 (half the fp32 bytes) and the
    per-slot results are stored as bf16 as well.

DMA orchestration (the measured constraints that shaped it):
  - Descriptor generation costs ~0.65us per dma_start (serial per HWDGE
    ring) regardless of size; consumers gate on DMA *completion* = last
    byte + ~1us HBM receipt.  So: staged small chunks for bank 0 (the first
    matmul gates on a 128KB completion), full 512KB slabs through the
    middle (gen-rate), halves at the end.
  - Bank 7 is split by OUT columns (each half = all kc chunks for 256 out
    columns) so the final DMA gates only 4 short matmuls + a half-size cast
    + a 16KB store instead of the whole bank's output.
  - The 8 DMAHW completion lanes are assigned round-robin in scheduled
    block order, and lane reuse waits on the previous user's completion:
    all weight DMAs are created contiguously and the stores are order-
    pinned after them so store lanes land on early-completing weight DMAs.
  - Weights ride the sync-ring HWDGE; y stores ride the scalar ring.
  - Each pair gets its own PSUM tiles (4 rotation rounds = 8 banks), so
    casts never gate the matmul stream.
  - Bias is folded in on the host; host scatter-adds the per-pair device
    results into the output.

Fixed shapes: B=2, T=256, K=2, IN=OUT=512, NB=64 banks, 8 cores.
Capacity: 32 slots/bank (binomial mean 16, sd ~4; overflow pairs — none for
realistic routing — are handled exactly on the host as a fallback).
"""

import numpy as np
from contextlib import ExitStack

B, T, KSEL = 2, 256, 2
IN, OUT, NB = 512, 512, 64
NCORES = 8
BPC = NB // NCORES          # banks per core = 8
CAP = 32                    # padded token slots per bank
SLOTS = BPC * CAP           # 256 dispatch rows per core
PCHUNK = 128                # contraction chunk (SBUF partition dim)
KC = IN // PCHUNK           # 4 contraction chunks
OHALF = OUT // 2            # bank-7 OUT-split half width

_cache = {}

# kc-split weight DMA chunking for banks 0..6 (bank 7 is OUT-split)
WCHUNKS = [[1, 1, 2], [4], [4], [4], [4], [4], [2, 2]]


def _build_nc():
    """Build the Bass/Tile program (one SPMD NeuronCore program)."""
    import concourse.tile as tile
    import concourse.mybir as mybir
    from concourse import bacc

    f32 = mybir.dt.float32
    bf16 = mybir.dt.bfloat16
    nc = bacc.Bacc("TRN2", target_bir_lowering=False, debug=False,
                   num_devices=NCORES)
    # host-pre-swizzled SBUF layouts: partition dim first, contiguous free
    # dim.  Bank 7's row of w holds its two OUT-halves back to back, each
    # [128, (kc, out_half)].
    xt = nc.dram_tensor("xt", [PCHUNK, KC * SLOTS], bf16,
                        kind="ExternalInput").ap()
    w = nc.dram_tensor("w", [BPC, PCHUNK, KC * OUT], bf16,
                       kind="ExternalInput").ap()
    y = nc.dram_tensor("y", [SLOTS - CAP, OUT], bf16,
                       kind="ExternalOutput").ap()
    y7 = nc.dram_tensor("y7", [2, CAP, OHALF], bf16,
                        kind="ExternalOutput").ap()

    from concourse.tile import add_dep_helper

    def chain(dep_chain, binst, reason):
        # pin scheduler order: binst depends on the previous link
        if dep_chain:
            add_dep_helper(binst.ins, dep_chain[-1].ins, sync=False,
                           reason=reason)
        dep_chain.append(binst)

    with tile.TileContext(nc) as tc:
        with ExitStack() as ctx:
            xpool = ctx.enter_context(tc.tile_pool(name="xp", bufs=1))
            all_chunks = [k for ch in WCHUNKS for k in ch]
            wpools = {
                kh: ctx.enter_context(
                    tc.tile_pool(name=f"wp{kh}",
                                 bufs=sum(1 for k in all_chunks if k == kh)))
                for kh in sorted(set(all_chunks))
            }
            wpool7 = ctx.enter_context(tc.tile_pool(name="wp7", bufs=2))
            ypool = ctx.enter_context(tc.tile_pool(name="yp", bufs=3))
            ypool6 = ctx.enter_context(tc.tile_pool(name="yp6", bufs=1))
            ypool7 = ctx.enter_context(tc.tile_pool(name="yp7", bufs=1))
            # pairs 0-2: psA+psB per rotation round (2 PSUM banks x 3)
            pspool = ctx.enter_context(
                tc.tile_pool(name="ps", bufs=3, space="PSUM"))
            # pair 3: own tiles (2 more banks; 8 total, zero reuse)
            pspool3 = ctx.enter_context(
                tc.tile_pool(name="ps3", bufs=1, space="PSUM"))

            # token dispatch first on the sync ring: every matmul needs it
            xt_sb = xpool.tile([PCHUNK, KC * SLOTS], bf16, tag="xt")

            wq = []    # sync-ring DMA chain (keeps FIFO = weight order)
            sq = []    # scalar-ring chain: y stores
            mq = []    # PE matmul chain (keeps bank order = arrival order)
            chain(wq, nc.sync.dma_start(xt_sb[:], xt[:]), "xt first")

            # All weight DMAs created contiguously (lane hygiene, above).
            # wtiles[j] = [(kc_start, kh, tile), ...] covering bank j (kc
            # split); bank 7 instead gets two OUT-half tiles.
            wtiles = []
            for j in range(BPC - 1):
                tiles = []
                kc0 = 0
                for kh in WCHUNKS[j]:
                    ks = slice(kc0 * OUT, (kc0 + kh) * OUT)
                    w_t = wpools[kh].tile([PCHUNK, kh * OUT], bf16,
                                          tag=f"w{kh}")
                    chain(wq, nc.sync.dma_start(w_t[:], w[j, :, ks]),
                          "weight ring order")
                    tiles.append((kc0, kh, w_t))
                    kc0 += kh
                wtiles.append(tiles)
            w7tiles = []
            for oc in range(2):
                w_t = wpool7.tile([PCHUNK, KC * OHALF], bf16, tag="w7")
                chain(wq, nc.sync.dma_start(
                    w_t[:], w[BPC - 1, :,
                              oc * KC * OHALF:(oc + 1) * KC * OHALF]),
                    "weight ring order")
                w7tiles.append(w_t)

            def mm_bank(j, out_ap, kc, ws_ap, tile_col):
                xs = slice(kc * SLOTS + j * CAP, kc * SLOTS + (j + 1) * CAP)
                mm = nc.tensor.matmul(
                    out_ap, xt_sb[:, xs], ws_ap,
                    start=(kc == 0), stop=(kc == KC - 1),
                    tile_position=(0, tile_col), skip_group_check=True)
                return mm

            # Pairs 0-2: even bank in PE column group 0 -> psA, odd bank in
            # column group 1 -> psB upper half (baseline-proven layout).
            # Bank-major matmul order: the even bank's weights always arrive
            # first, so its matmuls and cast overlap the odd bank's stream.
            for p in range(3):
                psA = pspool.tile([CAP, OUT], f32, tag="psA")
                psB = pspool.tile([2 * CAP, OUT], f32, tag="psB")
                outs = (psA[:], psB[CAP:2 * CAP, :])
                ysb = ypool.tile([2 * CAP, OUT], bf16, tag="y")
                for q in range(2):
                    j = 2 * p + q
                    for kc in range(KC):
                        kc0, kh, w_t = next(
                            t for t in wtiles[j]
                            if t[0] <= kc < t[0] + t[1])
                        mm = mm_bank(j, outs[q], kc,
                                     w_t[:, (kc - kc0) * OUT:
                                         (kc - kc0 + 1) * OUT], q * CAP)
                        if kc == 0 and q == 0:
                            chain(mq, mm, "pair compute order")
                    nc.vector.tensor_copy(ysb[q * CAP:(q + 1) * CAP, :],
                                          outs[q])
                ydma = nc.scalar.dma_start(
                    y[p * 2 * CAP:(p + 1) * 2 * CAP, :], ysb[:])
                if not sq:
                    add_dep_helper(ydma.ins, wq[-1].ins, sync=False,
                                   reason="stores after weight lane cycle")
                chain(sq, ydma, "y store order")

            # Pair 3: bank 6 (kc halves) + bank 7 (OUT halves).
            psA3 = pspool3.tile([CAP, OUT], f32, tag="psA3")
            psB7 = pspool3.tile([CAP, OUT], f32, tag="psB7")
            ysb6 = ypool6.tile([CAP, OUT], bf16, tag="y6")
            for kc in range(KC):
                kc0, kh, w_t = next(
                    t for t in wtiles[6] if t[0] <= kc < t[0] + t[1])
                mm = mm_bank(6, psA3[:], kc,
                             w_t[:, (kc - kc0) * OUT:(kc - kc0 + 1) * OUT],
                             0)
                if kc == 0:
                    chain(mq, mm, "pair compute order")
            nc.vector.tensor_copy(ysb6[:], psA3[:])
            chain(sq, nc.scalar.dma_start(y[6 * CAP:7 * CAP, :], ysb6[:]),
                  "y store order")
            for oc in range(2):
                ocs = slice(oc * OHALF, (oc + 1) * OHALF)
                for kc in range(KC):
                    mm_bank(7, psB7[:, ocs], kc,
                            w7tiles[oc][:, kc * OHALF:(kc + 1) * OHALF], 0)
                ysb7 = ypool7.tile([CAP, OHALF], bf16, tag=f"y7{oc}")
                nc.vector.tensor_copy(ysb7[:], psB7[:, ocs])
                chain(sq, nc.scalar.dma_start(y7[oc], ysb7[:]),
                      "y store order")
    nc.compile()
    return nc


def _get_nc():
    if "nc" not in _cache:
        _cache["nc"] = _build_nc()
    return _cache["nc"]


def _bf16(a32):
    import ml_dtypes
    return a32.astype(ml_dtypes.bfloat16)


def _swizzle_x(xt):
    """[IN, SLOTS] -> [128, KC*SLOTS] with free index (kc, slot)."""
    return np.ascontiguousarray(
        xt.reshape(KC, PCHUNK, SLOTS).transpose(1, 0, 2).reshape(
            PCHUNK, KC * SLOTS))


def _swizzle_w(w):
    """[BPC, IN, OUT] -> [BPC, 128, KC*OUT].

    Banks 0-6: free index (kc, out).  Bank 7: two OUT-halves back to back,
    each with free index (kc, out_half)."""
    sw = w.reshape(BPC, KC, PCHUNK, OUT).transpose(0, 2, 1, 3)
    out = np.empty((BPC, PCHUNK, KC * OUT), dtype=w.dtype)
    out[:BPC - 1] = sw[:BPC - 1].reshape(BPC - 1, PCHUNK, KC * OUT)
    w7 = sw[BPC - 1]                                   # [128, KC, OUT]
    halves = [w7[:, :, oc * OHALF:(oc + 1) * OHALF].reshape(
        PCHUNK, KC * OHALF) for oc in range(2)]
    out[BPC - 1] = np.concatenate(halves, axis=1)
    return np.ascontiguousarray(out)


def _route(X, sel, prob):
    """Group token-bank pairs by bank, build per-core dispatch arrays.

    Returns (slot_tok [NCORES,SLOTS] int64 (-1=pad), slot_p, overflow list
    of (token, bank, prob))."""
    NT = X.shape[0]
    pair_tok = np.repeat(np.arange(NT, dtype=np.int64), KSEL)
    pair_bank = sel.reshape(-1)
    pair_p = prob.reshape(-1)

    order = np.argsort(pair_bank, kind="stable")
    counts = np.bincount(pair_bank, minlength=NB)
    starts = np.concatenate(([0], np.cumsum(counts)))

    slot_tok = np.full((NCORES, SLOTS), -1, dtype=np.int64)
    slot_p = np.zeros((NCORES, SLOTS), dtype=np.float32)
    overflow = []
    for b in range(NB):
        c, j = divmod(b, BPC)
        s0, s1 = starts[b], starts[b + 1]
        take = min(s1 - s0, CAP)
        idx = order[s0:s0 + take]
        slot_tok[c, j * CAP: j * CAP + take] = pair_tok[idx]
        slot_p[c, j * CAP: j * CAP + take] = pair_p[idx]
        for i in order[s0 + take:s1]:
            overflow.append((int(pair_tok[i]), b, float(pair_p[i])))
    return slot_tok, slot_p, overflow


def _combine(ys, slot_tok, X, sel, prob, weights, bias, overflow):
    NT = X.shape[0]
    out = np.zeros((NT, OUT), dtype=np.float32)
    for c in range(NCORES):
        tok = slot_tok[c]
        valid = tok >= 0
        np.add.at(out, tok[valid], ys[c][valid])
    # bias term for every pair (device computes x @ W only)
    for k in range(KSEL):
        out += prob[:, k, None] * bias[sel[:, k]]
    # exact host fallback for capacity-overflow pairs (expected: none)
    for t, b, p in overflow:
        out[t] += p * (X[t] @ weights[b])
    return out


def _run_device(in_maps, trace=False, **kwargs):
    from concourse.bass_utils import run_bass_kernel_spmd
    return run_bass_kernel_spmd(_get_nc(), in_maps,
                                core_ids=list(range(NCORES)),
                                trace=trace, **kwargs)


def kernel(_trace=False, _bass_results=None, **inputs):
    tensor = np.asarray(inputs["tensor"], dtype=np.float32)
    sel = np.asarray(inputs["bank_selections"]).astype(np.int64)
    prob = np.asarray(inputs["bank_probabilities"], dtype=np.float32)
    weights = np.asarray(inputs["weights"], dtype=np.float32)
    bias = np.asarray(inputs["bias"], dtype=np.float32)

    NT = tensor.shape[0] * tensor.shape[1]
    X = tensor.reshape(NT, IN)
    sel2 = sel.reshape(NT, KSEL)
    prob2 = prob.reshape(NT, KSEL)

    slot_tok, slot_p, overflow = _route(X, sel2, prob2)

    in_maps = []
    for c in range(NCORES):
        tok = slot_tok[c]
        rows = X[np.where(tok >= 0, tok, 0)] * slot_p[c][:, None]
        xt = np.ascontiguousarray(rows.T)              # [IN, SLOTS] fp32
        w32 = weights[c * BPC:(c + 1) * BPC]           # (8, 512, 512) fp32
        in_maps.append({
            "xt": _swizzle_x(_bf16(xt)),
            "w": _swizzle_w(_bf16(w32)),
        })

    res = _run_device(in_maps, trace=_trace)
    if _bass_results is not None:
        _bass_results.append(res)

    ys = []
    for c in range(NCORES):
        yc = np.empty((SLOTS, OUT), dtype=np.float32)
        yc[:SLOTS - CAP] = res.results[c]["y"].astype(np.float32)
        y7 = res.results[c]["y7"].astype(np.float32)   # [2, CAP, OHALF]
        for oc in range(2):
            yc[SLOTS - CAP:, oc * OHALF:(oc + 1) * OHALF] = y7[oc]
        ys.append(yc)

    out = _combine(ys, slot_tok, X, sel2, prob2, weights, bias, overflow)
    return out.reshape(tensor.shape[0], tensor.shape[1], OUT)
